# revision 24
# baseline (speedup 1.0000x reference)
"""Trainium2 Bass kernel for nn_EquiEncoder (gnn_message_passing).

Strategy (8 NeuronCores, SPMD):
  - Atoms globally sorted by CG-bead (mapping), dealt to cores in contiguous
    chunks cut at bead boundaries -> per-core atom shard + disjoint bead window.
  - Directed edges sharded by owner of src atom, grouped by 128-atom src block,
    padded per block to a uniform tile count T (data-independent program).
  - The returned outputs (H, h) do not depend on the vector channels v/V
    (dead code in the reference), and only the first F columns of each 3F-wide
    MLP / dist-embedding output are needed.
  - Per layer: feature-major node MLP -> AllGather of per-node messages p0
    -> per-edge dma_gather of p0[dst] -> e0 via RBF matmul (K=17) -> msg =
    e0*p0g -> scatter-by-src via one-hot matmul into PSUM -> h update.
  - Contractive block per local atom; H accumulated via one-hot (mapping)
    matmuls into a per-core bead window; divided by counts at the end.
Host does index-only preprocessing (sorting, padding, layout); all floating
point math runs on device.
"""

import sys

import numpy as np

sys.path.insert(0, "/opt/trn_rl_repo")

# --- problem constants (hardcoded per spec) ---
N = 20000
M = 2000
E_HALF = 160000
F = 128
N_RBF = 16
N_CONV = 2
CUT_MSG = 5.0
CUT_CG = 20.0
EPS = 1e-15
C = 8  # cores

P = 128  # partitions
K17 = N_RBF + 1


def _wrap128(lin, ncols):
    """linear [ncols*128, ...] -> [128, ncols, ...] with j -> [j%128, j//128]"""
    lin = np.asarray(lin)
    return np.ascontiguousarray(
        lin.reshape(ncols, P, *lin.shape[1:]).swapaxes(0, 1)
    )


def _wrap16(lin):
    """linear int idxs -> [128, n/16] wrapped-16, replicated to 128 partitions"""
    a = lin.reshape(-1, 16).T.copy()  # [16, n/16]
    return np.ascontiguousarray(np.tile(a, (8, 1)))


def _prep(z, cg_z, mapping, nbr_list, xyz, cg_xyz, NA, NB, n_atoms, n_beads):
    """Host-side index preprocessing. Returns per-core input dicts + meta."""
    NAT = NA // P
    perm = np.argsort(mapping, kind="stable")
    msort = mapping[perm]

    cuts = [0]
    for i in range(1, C):
        ideal = i * n_atoms // C
        b = msort[min(ideal, n_atoms - 1)]
        pos = int(np.searchsorted(msort, b))
        cuts.append(pos)
    cuts.append(n_atoms)
    cuts = np.array(cuts)
    counts = np.diff(cuts)
    assert counts.max() <= NA, f"core atom count {counts.max()} > NA={NA}"

    bstart = []
    for i in range(C):
        bstart.append(int(msort[cuts[i]]) if counts[i] > 0 else (bstart[i - 1] if i else 0))
    bstart.append(n_beads)
    for i in range(C):
        assert bstart[i + 1] - bstart[i] <= NB, (
            f"bead window {bstart[i + 1] - bstart[i]} > NB={NB}"
        )

    newid = np.full(n_atoms, -1, np.int64)
    for i in range(C):
        newid[perm[cuts[i] : cuts[i + 1]]] = i * NA + np.arange(counts[i])

    src = np.concatenate([nbr_list[:, 0], nbr_list[:, 1]])
    dst = np.concatenate([nbr_list[:, 1], nbr_list[:, 0]])
    s_new = newid[src]
    d_new = newid[dst]

    order = np.lexsort((s_new,))
    src_o, dst_o = src[order], dst[order]
    s_o, d_o = s_new[order], d_new[order]

    key = (s_new // NA) * NAT + (s_new % NA) // P
    cnt_per_block = np.bincount(key, minlength=C * NAT)
    T = max(1, int(np.ceil(cnt_per_block.max() / P)))
    S = NAT * T

    per_core = []
    estart = np.concatenate([[0], np.cumsum(cnt_per_block)])
    for i in range(C):
        esr = np.full(S * P, -1.0, np.float32)
        edst = np.zeros(S * P, np.int64)
        exs = np.ones((S * P, 3), np.float32)
        exd = np.zeros((S * P, 3), np.float32)
        for b in range(NAT):
            k = i * NAT + b
            e0, e1 = estart[k], estart[k + 1]
            n_e = e1 - e0
            off = b * T * P
            esr[off : off + n_e] = ((s_o[e0:e1] % NA) % P).astype(np.float32)
            edst[off : off + n_e] = d_o[e0:e1]
            exs[off : off + n_e] = xyz[src_o[e0:e1]]
            exd[off : off + n_e] = xyz[dst_o[e0:e1]]
        n_i = counts[i]
        idx = perm[cuts[i] : cuts[i + 1]]
        zl = np.zeros(NA, np.int64)
        zl[:n_i] = z[idx]
        cl = np.zeros(NA, np.int64)
        cl[:n_i] = cg_z[mapping[idx]]
        ax = np.ones((NA, 3), np.float32)
        ax[:n_i] = xyz[idx]
        cp = np.zeros((NA, 3), np.float32)
        cp[:n_i] = cg_xyz[mapping[idx]]
        mr = np.full(NA, -1.0, np.float32)
        mr[:n_i] = (mapping[idx] - bstart[i]).astype(np.float32)
        cnt = np.bincount(mapping[idx] - bstart[i], minlength=NB).astype(np.float32)
        cnt = np.maximum(cnt[:NB], 1.0)

        exyz = np.concatenate([exs, exd], axis=1)  # [S*P, 6]
        axyz = np.concatenate([ax, cp], axis=1)  # [NA, 6]
        per_core.append(
            dict(
                esr=_wrap128(esr, S),
                edst=_wrap16(edst.astype(np.int16)),
                exyz=_wrap128(exyz, S),
                axyz=_wrap128(axyz, NAT),
                mrel=_wrap128(mr, NAT),
                zi=_wrap16(zl.astype(np.int16)),
                ci=_wrap16(cl.astype(np.int16)),
                cnt=np.ascontiguousarray(cnt.reshape(NB // P, P).T),
            )
        )

    meta = dict(perm=perm, cuts=cuts, counts=counts, bstart=bstart, T=T, S=S)
    return per_core, meta


def build_bass(NA, NB, T):
    """Build the SPMD Bass program."""
    from contextlib import ExitStack

    import concourse.bacc as bacc
    import concourse.mybir as mybir
    import concourse.tile as tile
    from concourse.masks import make_identity

    dt = mybir.dt
    Alu = mybir.AluOpType
    Act = mybir.ActivationFunctionType

    NAT = NA // P
    NBT = NB // P
    S = NAT * T
    NTOT = C * NA

    nc = bacc.Bacc(
        "TRN2", target_bir_lowering=False, debug=False, enable_asserts=False,
        num_devices=C,
    )

    f32 = dt.float32
    i16 = dt.int16

    def din(name, shape, d=f32):
        return nc.dram_tensor(name, list(shape), d, kind="ExternalInput")

    esr_d = din("esr", [P, S])
    edst_d = din("edst", [P, S * 8], i16)
    exyz_d = din("exyz", [P, S, 6])
    axyz_d = din("axyz", [P, NAT, 6])
    mrel_d = din("mrel", [P, NAT])
    zi_d = din("zi", [P, NA // 16], i16)
    ci_d = din("ci", [P, NA // 16], i16)
    cnt_d = din("cnt", [P, NBT])
    emb_atom_d = din("emb_atom", [100, 64])
    emb_res_d = din("emb_res", [30, 64])
    wm1_d = din("wm1", [N_CONV, F, F])
    bm1_d = din("bm1", [N_CONV, F, 1])
    wm2_d = din("wm2", [N_CONV, F, F])
    bm2_d = din("bm2", [N_CONV, F, 1])
    wdm_d = din("wdm", [N_CONV, K17, F])
    wc1_d = din("wc1", [N_CONV, F, F])
    bc1_d = din("bc1", [N_CONV, F, 1])
    wc2_d = din("wc2", [N_CONV, F, F])
    bc2_d = din("bc2", [N_CONV, F, 1])
    wdc_d = din("wdc", [N_CONV, K17, F])

    hout_d = nc.dram_tensor("h_out", [P, NA], f32, kind="ExternalOutput")
    Hout_d = nc.dram_tensor("H_out", [NB, F], f32, kind="ExternalOutput")

    NG = (S + 2) // 3
    NGC = (NAT + 2) // 3
    MW = min(512, NA)  # mlp chunk width

    with tile.TileContext(nc) as tc, ExitStack() as ex:
        con = ex.enter_context(tc.tile_pool(name="con", bufs=1))
        sb = ex.enter_context(tc.tile_pool(name="sb", bufs=3))
        ps = ex.enter_context(tc.tile_pool(name="ps", bufs=2, space="PSUM"))
        psh = ex.enter_context(tc.tile_pool(name="psh", bufs=1, space="PSUM"))
        gp = ex.enter_context(tc.tile_pool(name="gp", bufs=2))
        dram = ex.enter_context(tc.tile_pool(name="dram", bufs=1, space="DRAM"))
        bf16 = dt.bfloat16

        def pst(shape, name):
            return ps.tile(shape, f32, name=name, tag="a")

        ident = con.tile([P, P], f32, tag="ident")
        make_identity(nc, ident[:])
        ident_bf = con.tile([P, P], bf16, tag="ident_bf")
        nc.vector.tensor_copy(ident_bf[:], ident[:])
        i128i = con.tile([P, P], dt.int32, tag="i128i")
        nc.gpsimd.iota(i128i[:], pattern=[[1, P]], base=0, channel_multiplier=0)
        i128 = con.tile([P, P], f32, tag="i128")
        nc.vector.tensor_copy(i128[:], i128i[:])
        inbi = con.tile([P, NB], dt.int32, tag="inbi")
        nc.gpsimd.iota(inbi[:], pattern=[[1, NB]], base=0, channel_multiplier=0)
        inb = con.tile([P, NB], f32, tag="inb")
        nc.vector.tensor_copy(inb[:], inbi[:])
        b_eps = con.tile([P, 1], f32, tag="b_eps")
        nc.vector.memset(b_eps[:], 3.0 * EPS)
        b_npi = con.tile([P, 1], f32, tag="b_npi")
        nc.vector.memset(b_npi[:], float(-np.pi))

        def sin_rr(out_ap, d_ap, a, b, tmp_pool, tagp):
            """out = sin(a*d + b), range-reduced via r - cast(cast(r)) so tiny
            arguments keep full precision (self-edges have d ~ 5e-8) and the
            ISA only sees mul/add/compare (no mod)."""
            shp = list(d_ap.shape)
            twopi = 2.0 * np.pi
            t = tmp_pool.tile(shp, f32, name="srr_t", tag="srr")
            nc.vector.tensor_scalar(
                out=t[:], in0=d_ap, scalar1=float(a / twopi),
                scalar2=float(b / twopi), op0=Alu.mult, op1=Alu.add,
            )
            ti = tmp_pool.tile(shp, dt.int32, name="srr_i", tag="srri")
            nc.vector.tensor_copy(ti[:], t[:])
            tf = tmp_pool.tile(shp, f32, name="srr_f", tag="srrf")
            nc.vector.tensor_copy(tf[:], ti[:])
            nc.vector.tensor_tensor(out=t[:], in0=t[:], in1=tf[:], op=Alu.subtract)
            m = tmp_pool.tile(shp, f32, name="srr_m", tag="srrm")
            nc.vector.tensor_scalar(
                out=m[:], in0=t[:], scalar1=0.5, scalar2=-1.0,
                op0=Alu.is_gt, op1=Alu.mult,
            )
            nc.vector.tensor_tensor(out=t[:], in0=t[:], in1=m[:], op=Alu.add)
            nc.vector.tensor_scalar(
                out=m[:], in0=t[:], scalar1=-0.5, scalar2=1.0,
                op0=Alu.is_lt, op1=Alu.mult,
            )
            nc.vector.tensor_tensor(out=t[:], in0=t[:], in1=m[:], op=Alu.add)
            nc.scalar.activation(out_ap, t[:], Act.Sin, scale=float(twopi))

        def load(dten, shape, d=f32):
            t = con.tile(list(shape), d, tag=dten.name)
            nc.sync.dma_start(out=t[:], in_=dten[:])
            return t

        esr = load(esr_d, [P, S])
        edst = load(edst_d, [P, S * 8], i16)
        mrel = load(mrel_d, [P, NAT])
        zi = load(zi_d, [P, NA // 16], i16)
        ci = load(ci_d, [P, NA // 16], i16)
        cntt = load(cnt_d, [P, NBT])
        cinv = con.tile([P, NBT], f32, tag="cinv")
        nc.vector.reciprocal(cinv[:], cntt[:])

        def loadw(dten, shape, i):
            t = con.tile(list(shape), f32, tag=f"{dten.name}{i}")
            nc.sync.dma_start(out=t[:], in_=dten[i])
            return t

        wm1 = [loadw(wm1_d, [F, F], i) for i in range(N_CONV)]
        bm1 = [loadw(bm1_d, [F, 1], i) for i in range(N_CONV)]
        wm2 = [loadw(wm2_d, [F, F], i) for i in range(N_CONV)]
        bm2 = [loadw(bm2_d, [F, 1], i) for i in range(N_CONV)]
        wc1 = [loadw(wc1_d, [F, F], i) for i in range(N_CONV)]
        bc1 = [loadw(bc1_d, [F, 1], i) for i in range(N_CONV)]
        wc2 = [loadw(wc2_d, [F, F], i) for i in range(N_CONV)]
        bc2 = [loadw(bc2_d, [F, 1], i) for i in range(N_CONV)]

        # ---- stage 1: h0 embeddings (scoped pool, freed before geometry) ----
        emb_cm = tc.tile_pool(name="embp", bufs=1)
        embp = emb_cm.__enter__()
        embA = embp.tile([P, NAT, 64], f32, tag="embA")
        embB = embp.tile([P, NAT, 64], f32, tag="embB")
        for q in range(0, NAT, 4):  # <=512 descs per gather (ring limit)
            wq = min(4, NAT - q)
            nc.gpsimd.dma_gather(
                out_ap=embA[:, q : q + wq, :], in_ap=emb_atom_d[:],
                idxs_ap=zi[:, q * 8 : (q + wq) * 8],
                num_idxs=wq * P, num_idxs_reg=wq * P, elem_size=64,
            )
            nc.gpsimd.dma_gather(
                out_ap=embB[:, q : q + wq, :], in_ap=emb_res_d[:],
                idxs_ap=ci[:, q * 8 : (q + wq) * 8],
                num_idxs=wq * P, num_idxs_reg=wq * P, elem_size=64,
            )
        hT = con.tile([P, NA], f32, tag="hT")
        for c in range(NAT):
            for half, src_t in ((0, embA), (1, embB)):
                tp = pst([P, P], "tp_h0")
                nc.tensor.transpose(tp[:64, :], src_t[:, c, :], ident[:])
                nc.scalar.activation(
                    hT[half * 64 : half * 64 + 64, c * P : (c + 1) * P],
                    tp[:64, :], Act.Copy,
                )

        emb_cm.__exit__(None, None, None)

        # ---- stage 2: geometry/RBF tables (scoped pool) ----
        setup_cm = tc.tile_pool(name="setup", bufs=1)
        setup = setup_cm.__enter__()

        def sload(dten, shape, d=f32):
            t = setup.tile(list(shape), d, name=dten.name + "_s", tag=dten.name)
            nc.sync.dma_start(out=t[:], in_=dten[:])
            return t

        exyz = sload(exyz_d, [P, S, 6])
        axyz = sload(axyz_d, [P, NAT, 6])

        def geom(xyz6, ncols, cutoff, tagp):
            d2 = setup.tile([P, ncols], f32, name="g_d2", tag="g_d2")
            tmp = setup.tile([P, ncols], f32, name="g_tmp", tag="g_tmp")
            for k in range(3):
                nc.vector.tensor_tensor(
                    out=tmp[:], in0=xyz6[:, :, k], in1=xyz6[:, :, k + 3],
                    op=Alu.subtract,
                )
                if k == 0:
                    nc.vector.tensor_tensor(out=d2[:], in0=tmp[:], in1=tmp[:], op=Alu.mult)
                else:
                    t2 = setup.tile([P, ncols], f32, name="g_t2", tag="g_t2")
                    nc.vector.tensor_tensor(out=t2[:], in0=tmp[:], in1=tmp[:], op=Alu.mult)
                    nc.vector.tensor_tensor(out=d2[:], in0=d2[:], in1=t2[:], op=Alu.add)
            d = setup.tile([P, ncols], f32, name="g_d", tag=f"g_d{tagp}")
            nc.scalar.activation(d[:], d2[:], Act.Sqrt, bias=b_eps[:, 0:1], scale=1.0)
            c1 = setup.tile([P, ncols], f32, name="g_c1", tag="g_c1")
            sin_rr(c1[:], d[:], np.pi / cutoff, np.pi / 2, setup, "g" + tagp)
            env = setup.tile([P, ncols], f32, name="g_env", tag=f"g_env{tagp}")
            nc.vector.tensor_scalar(
                out=env[:], in0=c1[:], scalar1=0.5, scalar2=0.5,
                op0=Alu.mult, op1=Alu.add,
            )
            mask = setup.tile([P, ncols], f32, name="g_mask", tag="g_mask")
            nc.vector.tensor_scalar(
                out=mask[:], in0=d[:], scalar1=float(cutoff), scalar2=None, op0=Alu.is_lt
            )
            nc.vector.tensor_tensor(out=env[:], in0=env[:], in1=mask[:], op=Alu.mult)
            rec = setup.tile([P, ncols], f32, name="g_rec", tag="g_rec")
            nc.vector.reciprocal(rec[:], d[:])
            envd = setup.tile([P, ncols], f32, name="g_envd", tag=f"g_envd{tagp}")
            nc.vector.tensor_tensor(out=envd[:], in0=env[:], in1=rec[:], op=Alu.mult)
            return envd, env, d

        def build_lt(xyz6, ncols, cutoff, ngroups, tagp):
            """Pack per-tile lhsT [K17, 128] at partition bases 0/32/64 (3/group)."""
            envd, env, d = geom(xyz6, ncols, cutoff, tagp)
            sins = setup.tile([P, ncols, 32], bf16, name="sins", tag=f"sins{tagp}")
            nc.vector.memset(sins[:], 0.0)
            stmp = setup.tile([P, ncols], f32, name="stmp", tag="stmp")
            for k in range(N_RBF):
                nk = float((k + 1) * np.pi / cutoff)
                sin_rr(stmp[:], d[:], nk, 0.0, setup, tagp)
                nc.vector.tensor_tensor(
                    out=sins[:, :, k], in0=stmp[:], in1=envd[:], op=Alu.mult
                )
            nc.vector.tensor_copy(sins[:, :, N_RBF], env[:])
            lts = []
            for g in range(ngroups):
                w = min(3, ncols - g * 3)
                tp = ps.tile([P, P], bf16, name="tp_lt", tag="a")
                nc.tensor.transpose(
                    tp[: w * 32, :], sins[:, g * 3 : g * 3 + w, :], ident_bf[:]
                )
                lt = con.tile([P, P], bf16, name=f"lt{tagp}{g}", tag=f"lt{tagp}{g}")
                nc.scalar.activation(lt[: w * 32, :], tp[: w * 32, :], Act.Copy)
                lts.append(lt)
            return lts

        lte = build_lt(exyz, S, CUT_MSG, NG, "e")
        ltc = build_lt(axyz, NAT, CUT_CG, NGC, "c")

        # RBF weights replicated at partition bases 0/32/64 (rhs/lhsT base must
        # match the lt slice base)
        wdm3 = []
        wdc3 = []
        for i in range(N_CONV):
            for nm, dten, lst in (("wdm3", wdm_d, wdm3), ("wdc3", wdc_d, wdc3)):
                stg = setup.tile([K17, F], f32, name=f"{nm}stg{i}", tag=f"{nm}stg")
                nc.sync.dma_start(out=stg[:], in_=dten[i])
                t = con.tile([P, F], bf16, name=f"{nm}_{i}", tag=f"{nm}_{i}")
                for q in range(3):
                    nc.scalar.activation(t[q * 32 : q * 32 + K17, :], stg[:], Act.Copy)
                lst.append(t)

        setup_cm.__exit__(None, None, None)

        def mlp(out_t, in_t, w1, b1, w2, b2, tagp):
            y1 = sb.tile([P, NA], f32, name="y1", tag="y1", bufs=1)
            for q in range(NA // MW):
                sl = slice(q * MW, (q + 1) * MW)
                yp = pst([P, MW], "yp1")
                nc.tensor.matmul(yp[:], lhsT=w1[:], rhs=in_t[:, sl], start=True, stop=True)
                xb = sb.tile([P, MW], f32, name="xb", tag="xb", bufs=2)
                nc.scalar.activation(xb[:], yp[:], Act.Identity, bias=b1[:, 0:1], scale=1.0)
                sg = sb.tile([P, MW], f32, name="sg", tag="sg", bufs=2)
                nc.scalar.activation(sg[:], yp[:], Act.Sigmoid, bias=b1[:, 0:1], scale=1.0)
                nc.vector.tensor_tensor(out=y1[:, sl], in0=xb[:], in1=sg[:], op=Alu.mult)
            for q in range(NA // MW):
                sl = slice(q * MW, (q + 1) * MW)
                yp = pst([P, MW], "yp2")
                nc.tensor.matmul(yp[:], lhsT=w2[:], rhs=y1[:, sl], start=True, stop=True)
                nc.scalar.activation(
                    out_t[:, sl], yp[:], Act.Identity, bias=b2[:, 0:1], scale=1.0
                )

        Hacc = con.tile([P, NBT * F], f32, tag="Hacc")
        nc.vector.memset(Hacc[:], 0.0)

        def bead_scatter(rhs_fn):
            hp = [psh.tile([P, F], f32, name=f"hp{w}", tag=f"hp{w}") for w in range(NBT)]
            for c in range(NAT):
                rhs_t = rhs_fn(c)
                ohm = sb.tile([P, NB], f32, tag="ohm")
                nc.vector.tensor_scalar(
                    out=ohm[:], in0=inb[:], scalar1=mrel[:, c : c + 1],
                    scalar2=None, op0=Alu.is_equal,
                )
                for w in range(NBT):
                    nc.tensor.matmul(
                        hp[w][:], lhsT=ohm[:, w * P : (w + 1) * P], rhs=rhs_t,
                        start=(c == 0), stop=(c == NAT - 1), skip_group_check=True,
                    )
            for w in range(NBT):
                nc.vector.tensor_tensor(
                    out=Hacc[:, w * F : (w + 1) * F],
                    in0=Hacc[:, w * F : (w + 1) * F], in1=hp[w][:], op=Alu.add,
                )

        bounce = dram.tile([NA, F], f32, tag="bounce")
        gathered = dram.tile([NTOT, F], f32, tag="gathered")

        for i in range(N_CONV):
            p0T = sb.tile([P, NA], f32, name="p0T", tag="pT", bufs=1)
            mlp(p0T, hT, wm1[i], bm1[i], wm2[i], bm2[i], "m")
            p0n = sb.tile([P, NAT, F], f32, name="p0n", tag="p0n", bufs=1)
            for c in range(NAT):
                tp = pst([P, P], "tp_p0")
                nc.tensor.transpose(tp[:], p0T[:, c * P : (c + 1) * P], ident[:])
                nc.scalar.activation(p0n[:, c, :], tp[:], Act.Copy)
            nc.sync.dma_start(
                out=bounce[:].rearrange("(c p) f -> p c f", p=P), in_=p0n[:]
            )
            nc.gpsimd.collective_compute(
                "AllGather", Alu.bypass, replica_groups=[list(range(C))],
                ins=[bounce[:]], outs=[gathered[:]],
            )
            GC = min(T, 4)  # gather chunk (tiles); 8*128 descs <= ring carveout
            for b in range(NAT):
                p0gs = []
                for s0 in range(0, T, GC):
                    w = min(GC, T - s0)
                    p0g = gp.tile([P, GC, F], f32, name=f"p0g{s0}", tag="p0g", bufs=2)
                    base = b * T * 8 + s0 * (P // 16)
                    nc.gpsimd.dma_gather(
                        out_ap=p0g[:, :w, :], in_ap=gathered[:],
                        idxs_ap=edst[:, base : base + w * P // 16],
                        num_idxs=w * P, num_idxs_reg=w * P, elem_size=F,
                    )
                    p0gs.append(p0g)
                hup = ps.tile([P, P], f32, name="hup", tag="hup")
                for t in range(T):
                    c = b * T + t
                    p0g = p0gs[t // GC]
                    e0p = pst([P, F], "e0p")
                    g, r = c // 3, c % 3
                    nc.tensor.matmul(
                        e0p[:], lhsT=lte[g][r * 32 : r * 32 + K17, :],
                        rhs=wdm3[i][r * 32 : r * 32 + K17, :], start=True, stop=True,
                    )
                    msg = sb.tile([P, F], f32, tag="msg")
                    nc.vector.tensor_tensor(
                        out=msg[:], in0=e0p[:], in1=p0g[:, t % GC, :], op=Alu.mult
                    )
                    oh = sb.tile([P, P], f32, tag="oh")
                    nc.vector.tensor_scalar(
                        out=oh[:], in0=i128[:], scalar1=esr[:, c : c + 1],
                        scalar2=None, op0=Alu.is_equal,
                    )
                    nc.tensor.matmul(
                        hup[:], lhsT=msg[:], rhs=oh[:],
                        start=(t == 0), stop=(t == T - 1), skip_group_check=True,
                    )
                nc.vector.tensor_tensor(
                    out=hT[:, b * P : (b + 1) * P], in0=hT[:, b * P : (b + 1) * P],
                    in1=hup[:], op=Alu.add,
                )
            pc0T = sb.tile([P, NA], f32, name="pc0T", tag="pT", bufs=1)
            mlp(pc0T, hT, wc1[i], bc1[i], wc2[i], bc2[i], "c")

            def s0_rhs(c, i=i, pc0T=pc0T):
                ec0p = pst([P, F], "ec0p")
                g, r = c // 3, c % 3
                nc.tensor.matmul(
                    ec0p[:], lhsT=wdc3[i][r * 32 : r * 32 + K17, :],
                    rhs=ltc[g][r * 32 : r * 32 + K17, :], start=True, stop=True,
                )
                s0t = sb.tile([P, F], f32, tag="s0t")
                nc.vector.tensor_tensor(
                    out=s0t[:], in0=pc0T[:, c * P : (c + 1) * P], in1=ec0p[:],
                    op=Alu.mult,
                )
                tp = pst([P, P], "tp_s0")
                nc.tensor.transpose(tp[:], s0t[:], ident[:])
                s0n = sb.tile([P, F], f32, tag="s0n")
                nc.scalar.activation(s0n[:], tp[:], Act.Copy)
                return s0n[:]

            bead_scatter(s0_rhs)

            if i == 0:
                def h_rhs(c):
                    tp = pst([P, P], "tp_hn")
                    nc.tensor.transpose(tp[:], hT[:, c * P : (c + 1) * P], ident[:])
                    hn = sb.tile([P, F], f32, tag="hn")
                    nc.scalar.activation(hn[:], tp[:], Act.Copy)
                    return hn[:]

                bead_scatter(h_rhs)

        Hsc = sb.tile([P, NBT * F], f32, name="Hsc", tag="Hsc", bufs=1)
        for w in range(NBT):
            nc.vector.tensor_scalar(
                out=Hsc[:, w * F : (w + 1) * F], in0=Hacc[:, w * F : (w + 1) * F],
                scalar1=cinv[:, w : w + 1], scalar2=None, op0=Alu.mult,
            )
            nc.sync.dma_start(
                out=Hout_d[w * P : (w + 1) * P, :], in_=Hsc[:, w * F : (w + 1) * F]
            )
        nc.sync.dma_start(out=hout_d[:], in_=hT[:])

    nc.compile()
    return nc


def _weights_inputs(inputs):
    def a(x):
        return np.ascontiguousarray(np.asarray(x), dtype=np.float32)

    out = {}
    out["wm1"] = a(inputs["Wm1"])
    out["bm1"] = a(inputs["bm1"])[:, :F, None]
    out["wm2"] = a(np.asarray(inputs["Wm2"])[:, :, :F])
    out["bm2"] = a(inputs["bm2"])[:, :F, None]
    out["wdm"] = a(
        np.concatenate(
            [np.asarray(inputs["Wdm"])[:, :, :F], np.asarray(inputs["bdm"])[:, None, :F]],
            axis=1,
        )
    )
    out["wc1"] = a(inputs["Wc1"])
    out["bc1"] = a(inputs["bc1"])[:, :F, None]
    out["wc2"] = a(np.asarray(inputs["Wc2"])[:, :, :F])
    out["bc2"] = a(inputs["bc2"])[:, :F, None]
    out["wdc"] = a(
        np.concatenate(
            [np.asarray(inputs["Wdc"])[:, :, :F], np.asarray(inputs["bdc"])[:, None, :F]],
            axis=1,
        )
    )
    out["emb_atom"] = a(inputs["emb_atom"])
    out["emb_res"] = a(inputs["emb_res"])
    for k in ("bm1", "bm2", "bc1", "bc2"):
        out[k] = np.ascontiguousarray(out[k])
    return out


def assemble(results, meta, NA, NB, n_atoms, n_beads):
    """Host unshard (pure data movement)."""
    perm, cuts, counts, bstart = (
        meta["perm"], meta["cuts"], meta["counts"], meta["bstart"],
    )
    h_full = np.zeros((n_atoms, F), np.float32)
    H_full = np.zeros((n_beads, F), np.float32)
    for i in range(C):
        h_nm = results[i]["h_out"].T  # [NA, F] atom-major
        h_full[perm[cuts[i] : cuts[i + 1]]] = h_nm[: counts[i]]
        nb_i = bstart[i + 1] - bstart[i]
        H_full[bstart[i] : bstart[i + 1]] = results[i]["H_out"][:nb_i]
    return H_full, h_full


_CACHE = {}


def kernel(**inputs):
    from concourse.bass_utils import run_bass_kernel_spmd

    NA = 2560
    z = np.asarray(inputs["z"])
    cg_z = np.asarray(inputs["cg_z"])
    mapping = np.asarray(inputs["mapping"])
    nbr = np.asarray(inputs["nbr_list"])
    xyz = np.asarray(inputs["xyz"], dtype=np.float32)
    cg_xyz = np.asarray(inputs["cg_xyz"], dtype=np.float32)

    # adaptive bead-window size (multiple of 128)
    perm0 = np.argsort(mapping, kind="stable")
    msort0 = mapping[perm0]
    maxwin = 0
    prev = 0
    for i in range(1, C):
        pos = int(np.searchsorted(msort0, msort0[min(i * N // C, N - 1)]))
        maxwin = max(maxwin, int(msort0[pos - 1]) + 1 - int(msort0[prev]) if pos > prev else 0)
        prev = pos
    NB = 128
    per_core = meta = None
    for NB in (128, 256, 384, 512, 768, 1024):
        try:
            per_core, meta = _prep(z, cg_z, mapping, nbr, xyz, cg_xyz, NA, NB, N, M)
            break
        except AssertionError:
            continue
    T = meta["T"]

    key = (NA, NB, T)
    if key not in _CACHE:
        _CACHE[key] = build_bass(NA, NB, T)
    nc = _CACHE[key]

    wts = _weights_inputs(inputs)
    in_maps = [{**per_core[i], **wts} for i in range(C)]
    res = run_bass_kernel_spmd(nc, in_maps, core_ids=list(range(C)))
    return assemble(res.results, meta, NA, NB, N, M)
